# revision 21
# baseline (speedup 1.0000x reference)
"""Trainium2 Bass kernel for a 2-layer masked LSTM + FC + sigmoid head.

Problem shapes (hardcoded): B=1024, T=512, I=16, H=64.
Sharding: pure data parallel, batch 1024 -> 8 cores x 128.

Per-core design notes
---------------------
State layout is "transposed": hidden/cell states live as [H=64 partitions,
256 free] where free cols 0:128 = layer-0 batch, 128:256 = layer-1 batch.
The two LSTM layers are skewed by one timestep (at super-step k, layer 0
processes t=k while layer 1 processes t=k-1), so both layers' gate GEMMs
share one PSUM tile [128, 256] (gate rows on partitions, layer-blocks on
free) and all elementwise ops cover both layers in single instructions.

Masking: the reference freezes (h, c) where mask==0. Because the mask is a
length-prefix mask, the unmasked recurrence agrees with the masked one for
t < len(b), and the final layer-1 hidden equals h1 at t = len(b)-1. So we
run unmasked and accumulate h2_final = sum_t d_t * h1(t) with the one-hot
indicator d[b, t] = mask[b, t] - mask[b, t+1] (computed on host). d_t is
broadcast across the 64 H-partitions with a K=1 matmul into PSUM.

Biases (b_ih + b_hh, both layers) enter each gate PSUM tile through one
K=2 matmul: lhsT = [bias_L0; bias_L1] (2 x 128), rhs = layer-block select
mask (2 x 256).

x is pre-transposed on host into xs[p, c] = x[b, t, i] with
p = (t%4)*32 + i (i < 16; rows 16:32 of each group zero-padded so every
matmul rhs starts at a 32-aligned partition), c = (t//4)*128 + b.
"""

import os
from contextlib import ExitStack

import numpy as np

import concourse.bass as bass
import concourse.tile as tile
from concourse import bacc, mybir
from concourse import bass_utils

F32 = mybir.dt.float32
AF = mybir.ActivationFunctionType
OP = mybir.AluOpType

B, T, I, H = 1024, 512, 16, 64
NCORES = 8
BL = B // NCORES  # 128 batch per core
G4 = 4 * H  # 256

_BUILT = {}


def _build_program(t_steps: int):
    """Build the Bass program (single-core SPMD body). Returns compiled nc."""
    nc = bacc.Bacc(
        "TRN2",
        target_bir_lowering=False,
        debug=False,
        enable_asserts=False,
        num_devices=NCORES,
    )

    # ---- DRAM I/O ----
    d_xs = nc.dram_tensor("xs", [128, (t_steps // 4) * 128], F32, kind="ExternalInput")
    d_ds = nc.dram_tensor("ds", [128, (t_steps // 4) * 128], F32, kind="ExternalInput")
    d_w = {}
    for name, k in [
        ("wif0", 128), ("wog0", 128),
        ("whif0", 64), ("whog0", 64),
        ("wif1", 64), ("wog1", 64),
        ("whif1", 64), ("whog1", 64),
    ]:
        d_w[name] = nc.dram_tensor(name, [k, 128], F32, kind="ExternalInput")
    d_bif = nc.dram_tensor("bif", [2, 128], F32, kind="ExternalInput")
    d_bog = nc.dram_tensor("bog", [2, 128], F32, kind="ExternalInput")
    d_sel = nc.dram_tensor("sel", [2, 256], F32, kind="ExternalInput")
    d_sell1 = nc.dram_tensor("sell1", [2, 256], F32, kind="ExternalInput")
    d_fct = nc.dram_tensor("fct", [64, 1], F32, kind="ExternalInput")
    d_fcb = nc.dram_tensor("fcb", [1, 1], F32, kind="ExternalInput")
    d_out = nc.dram_tensor("out", [1, 128], F32, kind="ExternalOutput")

    with tile.TileContext(nc) as tc, ExitStack() as ctx:
        pconst = ctx.enter_context(tc.tile_pool(name="const", bufs=1))
        pstate = ctx.enter_context(tc.tile_pool(name="state", bufs=3))
        ppsum = ctx.enter_context(tc.tile_pool(name="psum", bufs=2, space="PSUM"))
        ppsd = ctx.enter_context(tc.tile_pool(name="psd", bufs=2, space="PSUM"))
        pwork = ctx.enter_context(tc.tile_pool(name="work", bufs=3))

        # ---- persistent SBUF: inputs ----
        xs = pconst.tile([128, (t_steps // 4) * 128], F32, tag="xs")
        n_xchunks = 8
        xw = (t_steps // 4) * 128 // n_xchunks
        for j in range(n_xchunks):
            nc.sync.dma_start(xs[:, j * xw:(j + 1) * xw], d_xs.ap()[:, j * xw:(j + 1) * xw])
        ds = pconst.tile([128, (t_steps // 4) * 128], F32, tag="ds")
        for j in range(4):
            dw = (t_steps // 4) * 128 // 4
            nc.sync.dma_start(ds[:, j * dw:(j + 1) * dw], d_ds.ap()[:, j * dw:(j + 1) * dw])

        w = {}
        for name, k in [
            ("wif0", 128), ("wog0", 128),
            ("whif0", 64), ("whog0", 64),
            ("wif1", 64), ("wog1", 64),
            ("whif1", 64), ("whog1", 64),
        ]:
            w[name] = pconst.tile([k, 128], F32, tag=name, name=name)
            nc.sync.dma_start(w[name][:], d_w[name].ap()[:])
        bif = pconst.tile([2, 128], F32, tag="bif")
        nc.sync.dma_start(bif[:], d_bif.ap()[:])
        bog = pconst.tile([2, 128], F32, tag="bog")
        nc.sync.dma_start(bog[:], d_bog.ap()[:])
        sel = pconst.tile([2, 256], F32, tag="sel")
        nc.sync.dma_start(sel[:], d_sel.ap()[:])
        sell1 = pconst.tile([2, 256], F32, tag="sell1")
        nc.sync.dma_start(sell1[:], d_sell1.ap()[:])
        fct = pconst.tile([64, 1], F32, tag="fct")
        nc.sync.dma_start(fct[:], d_fct.ap()[:])
        fcb = pconst.tile([1, 1], F32, tag="fcb")
        nc.sync.dma_start(fcb[:], d_fcb.ap()[:])

        ones4 = pconst.tile([128, 64], F32, tag="ones4")
        nc.vector.memset(ones4[:], 1.0)

        # ---- persistent SBUF: state ----
        # sh: [64, 256] = [h0 | h1]; gc: [128, 256] rows 0:64 = tanh'd g gate
        # (written each step), rows 64:128 = cell state c (carried).
        sh_prev = pstate.tile([64, 256], F32, tag="sh")
        nc.vector.memset(sh_prev[:], 0.0)
        gc_cur = pstate.tile([128, 256], F32, tag="gc")
        nc.vector.memset(gc_cur[:], 0.0)
        h2acc = pconst.tile([64, 128], F32, tag="h2acc")
        nc.vector.memset(h2acc[:], 0.0)

        for k in range(t_steps + 1):
            do_l0 = k < t_steps
            do_l1 = k >= 1

            # ---------- gate GEMMs ----------
            p_if = ppsum.tile([128, 256], F32, tag="pif")
            p_og = ppsum.tile([128, 256], F32, tag="pog")

            if do_l0:
                nc.tensor.matmul(p_if[:], bif[:], sel[:], start=True, stop=False)
                nc.tensor.matmul(p_og[:], bog[:], sel[:], start=True, stop=False)
                # stage x_t to a base-0 tile (GPSIMD, idle engine) so every
                # matmul in this PSUM accumulation group uses PE row-group 0:
                # mixing row-groups inside one group wedges the device.
                ph = (k % 4) * 32
                xsl = xs[ph:ph + 32, (k // 4) * 128:(k // 4) * 128 + 128]
                xst = pwork.tile([32, 128], F32, tag="xst")
                nc.gpsimd.tensor_copy(xst[:], xsl)
                nc.tensor.matmul(p_if[:, 0:128], w["wif0"][0:32, :], xst[:],
                                 start=False, stop=False)
                nc.tensor.matmul(p_og[:, 0:128], w["wog0"][0:32, :], xst[:],
                                 start=False, stop=False)
                nc.tensor.matmul(p_if[:, 0:128], w["whif0"][:], sh_prev[:, 0:128],
                                 start=False, stop=not do_l1)
                nc.tensor.matmul(p_og[:, 0:128], w["whog0"][:], sh_prev[:, 0:128],
                                 start=False, stop=not do_l1)
            else:
                nc.tensor.matmul(p_if[:, 128:256], bif[:], sell1[:, 128:256],
                                 start=True, stop=False)
                nc.tensor.matmul(p_og[:, 128:256], bog[:], sell1[:, 128:256],
                                 start=True, stop=False)
            if do_l1:
                nc.tensor.matmul(p_if[:, 128:256], w["wif1"][:], sh_prev[:, 0:128],
                                 start=False, stop=False)
                nc.tensor.matmul(p_og[:, 128:256], w["wog1"][:], sh_prev[:, 0:128],
                                 start=False, stop=False)
                nc.tensor.matmul(p_if[:, 128:256], w["whif1"][:], sh_prev[:, 128:256],
                                 start=False, stop=True)
                nc.tensor.matmul(p_og[:, 128:256], w["whog1"][:], sh_prev[:, 128:256],
                                 start=False, stop=True)

            # ---------- activations + cell/hidden update ----------
            # last iteration only computes the layer-1 column block
            c0 = 0 if do_l0 else 128
            g_if = pwork.tile([128, 256], F32, tag="gif")
            nc.scalar.activation(g_if[:, c0:256], p_if[:, c0:256], AF.Sigmoid)
            g_o = pwork.tile([64, 256], F32, tag="go")
            nc.scalar.activation(g_o[:, c0:256], p_og[0:64, c0:256], AF.Sigmoid)
            gc_next = pstate.tile([128, 256], F32, tag="gc")
            nc.scalar.activation(gc_next[0:64, c0:256], p_og[64:128, c0:256], AF.Tanh)

            # igfc cols 0:256 = i*g, cols 256:512 = f*c_prev (free-packed so the
            # add below sees two SBUF inputs at the same base partition).
            igfc = pwork.tile([64, 512], F32, tag="igfc")
            nc.vector.tensor_tensor(igfc[:, c0:256], g_if[0:64, c0:256],
                                    gc_next[0:64, c0:256], OP.mult)
            nc.vector.tensor_tensor(igfc[:, 256 + c0:512], g_if[64:128, c0:256],
                                    gc_cur[64:128, c0:256], OP.mult)
            # c_new -> gc_next rows 64:128 (used by next super-step)
            nc.vector.tensor_tensor(gc_next[64:128, c0:256], igfc[:, c0:256],
                                    igfc[:, 256 + c0:512], OP.add)
            tanh_c = pwork.tile([64, 256], F32, tag="tanhc")
            nc.scalar.activation(tanh_c[:, c0:256], gc_next[64:128, c0:256], AF.Tanh)
            sh_new = pstate.tile([64, 256], F32, tag="sh")
            nc.vector.tensor_tensor(sh_new[:, c0:256], g_o[:, c0:256],
                                    tanh_c[:, c0:256], OP.mult)

            if k == 0:
                # layer-1 columns ran on garbage (t=-1); reset to initial zeros
                nc.vector.memset(sh_new[:, 128:256], 0.0)
                nc.vector.memset(gc_next[64:128, 128:256], 0.0)

            # ---------- final-step capture for layer 1 (t = k-1) ----------
            if do_l1:
                tcap = k - 1
                pc = (tcap % 4) * 32
                dsl = ds[pc:pc + 1, (tcap // 4) * 128:(tcap // 4) * 128 + 128]
                psd = ppsd.tile([64, 128], F32, tag="psd")
                nc.tensor.matmul(psd[:], ones4[pc:pc + 1, :], dsl, start=True, stop=True,
                                 tile_position=(pc, 0))
                cap = pwork.tile([64, 128], F32, tag="cap")
                nc.vector.tensor_tensor(cap[:], psd[:], sh_new[:, 128:256], OP.mult)
                nc.vector.tensor_tensor(h2acc[:], h2acc[:], cap[:], OP.add)

            sh_prev = sh_new
            gc_cur = gc_next

        # ---------- FC + sigmoid head ----------
        pfc = ppsd.tile([1, 128], F32, tag="pfc")
        nc.tensor.matmul(pfc[:], fct[:], h2acc[:], start=True, stop=True)
        osb = pwork.tile([1, 128], F32, tag="osb")
        nc.scalar.activation(osb[:], pfc[:], AF.Sigmoid, bias=fcb[:, 0:1])
        nc.sync.dma_start(d_out.ap()[:], osb[:])

    nc.compile()
    return nc


def _get_program(t_steps: int):
    if t_steps not in _BUILT:
        _BUILT[t_steps] = _build_program(t_steps)
    return _BUILT[t_steps]


def _prep_core_inputs(x, dmask, weights, t_steps):
    """Host-side layout prep for one core's shard. x: [BL, T, I], dmask: [BL, T]."""
    tq = t_steps // 4
    # xs[p, c] = x[b, t, i] at p=(t%4)*32+i, c=(t//4)*128+b
    xpad = np.zeros((BL, t_steps, 32), np.float32)
    xpad[:, :, :I] = x
    xs = (
        xpad.transpose(1, 2, 0)           # [t, i32, b]
        .reshape(tq, 4, 32, BL)
        .transpose(1, 2, 0, 3)            # [t%4, i32, t//4, b]
        .reshape(128, tq * 128)
    )
    xs = np.ascontiguousarray(xs)
    # ds[p, c] = d[b, t] at p=(t%4)*32, c=(t//4)*128+b
    dsb = np.zeros((128, tq * 128), np.float32)
    dv = (
        dmask.transpose(1, 0)             # [t, b]
        .reshape(tq, 4, BL)
        .transpose(1, 0, 2)               # [t%4, t//4, b]
        .reshape(4, tq * 128)
    )
    dsb[0::32][:4] = dv
    return dict(xs=xs, ds=dsb, **weights)


def _host_weights(w_ih0, w_hh0, b_ih0, b_hh0,
                  w_ih1, w_hh1, b_ih1, b_hh1, fc_w, fc_b):
    def lt(a):  # lhsT helper
        return np.ascontiguousarray(np.asarray(a, np.float32).T)

    def pad32x4(a16):  # [16, 128] -> [128, 128]: zero-pad to 32 rows, tile 4x
        out = np.zeros((32, 128), np.float32)
        out[:16] = a16
        return np.tile(out, (4, 1))

    b0 = np.asarray(b_ih0, np.float32) + np.asarray(b_hh0, np.float32)
    b1 = np.asarray(b_ih1, np.float32) + np.asarray(b_hh1, np.float32)

    def og(a):  # reorder [4H, K] gate rows -> [o; g] stacked
        return np.concatenate([a[3 * H:4 * H], a[2 * H:3 * H]], axis=0)

    sel_row0 = np.concatenate([np.ones((1, 128), np.float32),
                               np.zeros((1, 128), np.float32)], axis=1)
    sel_row1 = np.concatenate([np.zeros((1, 128), np.float32),
                               np.ones((1, 128), np.float32)], axis=1)
    weights = dict(
        wif0=pad32x4(lt(np.asarray(w_ih0)[0:2 * H])),
        wog0=pad32x4(lt(og(np.asarray(w_ih0)))),
        whif0=lt(np.asarray(w_hh0)[0:2 * H]),
        whog0=lt(og(np.asarray(w_hh0))),
        wif1=lt(np.asarray(w_ih1)[0:2 * H]),
        wog1=lt(og(np.asarray(w_ih1))),
        whif1=lt(np.asarray(w_hh1)[0:2 * H]),
        whog1=lt(og(np.asarray(w_hh1))),
        bif=np.stack([b0[0:2 * H], b1[0:2 * H]]).astype(np.float32),
        bog=np.stack([
            np.concatenate([b0[3 * H:4 * H], b0[2 * H:3 * H]]),
            np.concatenate([b1[3 * H:4 * H], b1[2 * H:3 * H]]),
        ]).astype(np.float32),
        sel=np.concatenate([sel_row0, sel_row1]).astype(np.float32),
        sell1=np.concatenate([np.zeros((1, 256), np.float32), sel_row1]).astype(np.float32),
        fct=np.ascontiguousarray(np.asarray(fc_w, np.float32).reshape(1, H).T),
        fcb=np.asarray(fc_b, np.float32).reshape(1, 1),
    )
    return weights


def _run(x, mask, w_ih0, w_hh0, b_ih0, b_hh0,
         w_ih1, w_hh1, b_ih1, b_hh1, fc_w, fc_b, trace=False):
    t_steps = x.shape[1]
    x = np.asarray(x, np.float32)
    mask = np.asarray(mask)

    # d[b, t] = mask[b, t] - mask[b, t+1]  (one-hot at t = len_b - 1)
    m = mask.astype(np.float32)
    d = m - np.concatenate([m[:, 1:], np.zeros((m.shape[0], 1), np.float32)], axis=1)

    weights = _host_weights(w_ih0, w_hh0, b_ih0, b_hh0,
                            w_ih1, w_hh1, b_ih1, b_hh1, fc_w, fc_b)

    nc = _get_program(t_steps)
    in_maps = []
    for c in range(NCORES):
        sl = slice(c * BL, (c + 1) * BL)
        in_maps.append(_prep_core_inputs(x[sl], d[sl], weights, t_steps))

    res = bass_utils.run_bass_kernel_spmd(nc, in_maps, core_ids=list(range(NCORES)),
                                          trace=trace)
    out = np.concatenate([res.results[c]["out"].reshape(BL) for c in range(NCORES)])
    return out.astype(np.float32), res


def kernel(**inputs):
    return _run(**inputs)[0]


def kernel_traced(**inputs):
    return _run(**inputs, trace=True)


# revision 25
# speedup vs baseline: 1.3548x; 1.3548x over previous
"""Trainium2 Bass kernel for a 2-layer masked LSTM + FC + sigmoid head.

Problem shapes (hardcoded): B=1024, T=512, I=16, H=64.
Sharding: pure data parallel, batch 1024 -> 8 cores x 128.

Per-core design notes
---------------------
State layout is "transposed": hidden/cell states live as [H=64 partitions,
256 free] where free cols 0:128 = layer-0 batch, 128:256 = layer-1 batch.
The two LSTM layers are skewed by one timestep (at super-step k, layer 0
processes t=k while layer 1 processes t=k-1), so both layers' gate GEMMs
share one PSUM tile [128, 256] (gate rows on partitions, layer-blocks on
free) and all elementwise ops cover both layers in single instructions.

Masking: the reference freezes (h, c) where mask==0. Because the mask is a
length-prefix mask, the unmasked recurrence agrees with the masked one for
t < len(b), and the final layer-1 hidden equals h1 at t = len(b)-1. So we
run unmasked and accumulate h2_final = sum_t d_t * h1(t) with the one-hot
indicator d[b, t] = mask[b, t] - mask[b, t+1] (computed on host). d_t is
broadcast across the 64 H-partitions with a K=1 matmul into PSUM.

Biases (b_ih + b_hh, both layers) enter each gate PSUM tile through one
K=2 matmul: lhsT = [bias_L0; bias_L1] (2 x 128), rhs = layer-block select
mask (2 x 256).

x is pre-transposed on host into xs[p, c] = x[b, t, i] with
p = (t%4)*32 + i (i < 16; rows 16:32 of each group zero-padded so every
matmul rhs starts at a 32-aligned partition), c = (t//4)*128 + b.
"""

import os
from contextlib import ExitStack

import numpy as np

import concourse.bass as bass
import concourse.tile as tile
from concourse import bacc, mybir
from concourse import bass_utils

F32 = mybir.dt.float32
F32R = mybir.dt.float32r
AF = mybir.ActivationFunctionType
OP = mybir.AluOpType

B, T, I, H = 1024, 512, 16, 64
NCORES = 8
BL = B // NCORES  # 128 batch per core
G4 = 4 * H  # 256

_BUILT = {}


def _build_program(t_steps: int):
    """Build the Bass program (single-core SPMD body). Returns compiled nc."""
    nc = bacc.Bacc(
        "TRN2",
        target_bir_lowering=False,
        debug=False,
        enable_asserts=False,
        num_devices=NCORES,
    )

    # ---- DRAM I/O ----
    d_xs = nc.dram_tensor("xs", [128, (t_steps // 4) * 128], F32R, kind="ExternalInput")
    d_ds = nc.dram_tensor("ds", [128, (t_steps // 4) * 128], F32R, kind="ExternalInput")
    d_w = {}
    for name, k in [
        ("wif0", 128), ("wog0", 128),
        ("whif0", 64), ("whog0", 64),
        ("wif1", 64), ("wog1", 64),
        ("whif1", 64), ("whog1", 64),
    ]:
        d_w[name] = nc.dram_tensor(name, [k, 128], F32R, kind="ExternalInput")
    d_bif = nc.dram_tensor("bif", [2, 128], F32R, kind="ExternalInput")
    d_bog = nc.dram_tensor("bog", [2, 128], F32R, kind="ExternalInput")
    d_sel = nc.dram_tensor("sel", [2, 256], F32R, kind="ExternalInput")
    d_sell1 = nc.dram_tensor("sell1", [2, 256], F32R, kind="ExternalInput")
    d_fct = nc.dram_tensor("fct", [64, 1], F32R, kind="ExternalInput")
    d_ones4 = nc.dram_tensor("ones4", [128, 64], F32R, kind="ExternalInput")
    d_zini = nc.dram_tensor("zini", [64, 256], F32R, kind="ExternalInput")
    d_fcb = nc.dram_tensor("fcb", [1, 1], F32, kind="ExternalInput")
    d_out = nc.dram_tensor("out", [1, 128], F32, kind="ExternalOutput")

    def mm(out, lhsT, rhs, **kw):
        # fp32 matmul runs as 2 HW passes (~273ns); fp32r is 1 pass (~134ns)
        if lhsT.dtype == F32:
            lhsT = lhsT.bitcast(F32R)
        if rhs.dtype == F32:
            rhs = rhs.bitcast(F32R)
        return nc.tensor.matmul(out, lhsT, rhs, **kw)

    with tile.TileContext(nc) as tc, ExitStack() as ctx:
        pconst = ctx.enter_context(tc.tile_pool(name="const", bufs=1))
        pstate = ctx.enter_context(tc.tile_pool(name="state", bufs=3))
        ppsum = ctx.enter_context(tc.tile_pool(name="psum", bufs=2, space="PSUM"))
        ppsd = ctx.enter_context(tc.tile_pool(name="psd", bufs=2, space="PSUM"))
        pwork = ctx.enter_context(tc.tile_pool(name="work", bufs=3))

        # ---- persistent SBUF: inputs ----
        xs = pconst.tile([128, (t_steps // 4) * 128], F32R, tag="xs")
        n_xchunks = 8
        xw = (t_steps // 4) * 128 // n_xchunks
        for j in range(n_xchunks):
            nc.sync.dma_start(xs[:, j * xw:(j + 1) * xw], d_xs.ap()[:, j * xw:(j + 1) * xw])
        ds = pconst.tile([128, (t_steps // 4) * 128], F32R, tag="ds")
        for j in range(4):
            dw = (t_steps // 4) * 128 // 4
            nc.sync.dma_start(ds[:, j * dw:(j + 1) * dw], d_ds.ap()[:, j * dw:(j + 1) * dw])

        w = {}
        for name, k in [
            ("wif0", 128), ("wog0", 128),
            ("whif0", 64), ("whog0", 64),
            ("wif1", 64), ("wog1", 64),
            ("whif1", 64), ("whog1", 64),
        ]:
            w[name] = pconst.tile([k, 128], F32R, tag=name, name=name)
            nc.sync.dma_start(w[name][:], d_w[name].ap()[:])
        bif = pconst.tile([2, 128], F32R, tag="bif")
        nc.sync.dma_start(bif[:], d_bif.ap()[:])
        bog = pconst.tile([2, 128], F32R, tag="bog")
        nc.sync.dma_start(bog[:], d_bog.ap()[:])
        sel = pconst.tile([2, 256], F32R, tag="sel")
        nc.sync.dma_start(sel[:], d_sel.ap()[:])
        sell1 = pconst.tile([2, 256], F32R, tag="sell1")
        nc.sync.dma_start(sell1[:], d_sell1.ap()[:])
        fct = pconst.tile([64, 1], F32R, tag="fct")
        nc.sync.dma_start(fct[:], d_fct.ap()[:])
        fcb = pconst.tile([1, 1], F32, tag="fcb")
        nc.sync.dma_start(fcb[:], d_fcb.ap()[:])

        ones4 = pconst.tile([128, 64], F32R, tag="ones4")
        nc.sync.dma_start(ones4[:], d_ones4.ap()[:])

        # ---- persistent SBUF: state ----
        # sh: [64, 256] = [h0 | h1]; gc: [128, 256] rows 0:64 = tanh'd g gate
        # (written each step), rows 64:128 = cell state c (carried).
        sh_prev = pstate.tile([64, 256], F32R, tag="sh")
        nc.sync.dma_start(sh_prev[:], d_zini.ap()[:])
        gc_cur = pstate.tile([128, 256], F32, tag="gc")
        nc.vector.memset(gc_cur[:], 0.0)
        h2acc = pconst.tile([64, 128], F32R, tag="h2acc")
        nc.sync.dma_start(h2acc[:], d_zini.ap()[:, 0:128])

        for k in range(t_steps + 1):
            do_l0 = k < t_steps
            do_l1 = k >= 1

            # ---------- gate GEMMs ----------
            p_if = ppsum.tile([128, 256], F32, tag="pif")
            p_og = ppsum.tile([128, 256], F32, tag="pog")

            if do_l0:
                mm(p_if[:], bif[:], sel[:], start=True, stop=False)
                mm(p_og[:], bog[:], sel[:], start=True, stop=False)
                # stage x_t to a base-0 tile (GPSIMD, idle engine) so every
                # matmul in this PSUM accumulation group uses PE row-group 0:
                # mixing row-groups inside one group wedges the device.
                ph = (k % 4) * 32
                xsl = xs[ph:ph + 32, (k // 4) * 128:(k // 4) * 128 + 128]
                xst = pwork.tile([32, 128], F32R, tag="xst")
                nc.gpsimd.tensor_copy(xst[:], xsl)
                mm(p_if[:, 0:128], w["wif0"][0:32, :], xst[:],
                                 start=False, stop=False)
                mm(p_og[:, 0:128], w["wog0"][0:32, :], xst[:],
                                 start=False, stop=False)
                mm(p_if[:, 0:128], w["whif0"][:], sh_prev[:, 0:128],
                                 start=False, stop=not do_l1)
                mm(p_og[:, 0:128], w["whog0"][:], sh_prev[:, 0:128],
                                 start=False, stop=not do_l1)
            else:
                mm(p_if[:, 128:256], bif[:], sell1[:, 128:256],
                                 start=True, stop=False)
                mm(p_og[:, 128:256], bog[:], sell1[:, 128:256],
                                 start=True, stop=False)
            if do_l1:
                mm(p_if[:, 128:256], w["wif1"][:], sh_prev[:, 0:128],
                                 start=False, stop=False)
                mm(p_og[:, 128:256], w["wog1"][:], sh_prev[:, 0:128],
                                 start=False, stop=False)
                mm(p_if[:, 128:256], w["whif1"][:], sh_prev[:, 128:256],
                                 start=False, stop=True)
                mm(p_og[:, 128:256], w["whog1"][:], sh_prev[:, 128:256],
                                 start=False, stop=True)

            # ---------- activations + cell/hidden update ----------
            # last iteration only computes the layer-1 column block
            c0 = 0 if do_l0 else 128
            g_if = pwork.tile([128, 256], F32, tag="gif")
            nc.scalar.activation(g_if[:, c0:256], p_if[:, c0:256], AF.Sigmoid)
            g_o = pwork.tile([64, 256], F32, tag="go")
            nc.scalar.activation(g_o[:, c0:256], p_og[0:64, c0:256], AF.Sigmoid)
            gc_next = pstate.tile([128, 256], F32, tag="gc")
            nc.scalar.activation(gc_next[0:64, c0:256], p_og[64:128, c0:256], AF.Tanh)

            # igfc cols 0:256 = i*g, cols 256:512 = f*c_prev (free-packed so the
            # add below sees two SBUF inputs at the same base partition).
            igfc = pwork.tile([64, 512], F32, tag="igfc")
            nc.vector.tensor_tensor(igfc[:, c0:256], g_if[0:64, c0:256],
                                    gc_next[0:64, c0:256], OP.mult)
            nc.vector.tensor_tensor(igfc[:, 256 + c0:512], g_if[64:128, c0:256],
                                    gc_cur[64:128, c0:256], OP.mult)
            # c_new -> gc_next rows 64:128 (used by next super-step)
            nc.vector.tensor_tensor(gc_next[64:128, c0:256], igfc[:, c0:256],
                                    igfc[:, 256 + c0:512], OP.add)
            tanh_c = pwork.tile([64, 256], F32, tag="tanhc")
            nc.scalar.activation(tanh_c[:, c0:256], gc_next[64:128, c0:256], AF.Tanh)
            sh_new = pstate.tile([64, 256], F32R, tag="sh")
            nc.vector.tensor_tensor(sh_new[:, c0:256], g_o[:, c0:256],
                                    tanh_c[:, c0:256], OP.mult)

            if k == 0:
                # layer-1 columns ran on garbage (t=-1); reset to initial zeros
                # (DMA, not memset: DVE memset can't emit the f32r dtype)
                nc.sync.dma_start(sh_new[:, 128:256], d_zini.ap()[:, 0:128])
                nc.vector.memset(gc_next[64:128, 128:256], 0.0)

            # ---------- final-step capture for layer 1 (t = k-1) ----------
            if do_l1:
                tcap = k - 1
                pc = (tcap % 4) * 32
                dsl = ds[pc:pc + 1, (tcap // 4) * 128:(tcap // 4) * 128 + 128]
                psd = ppsd.tile([64, 128], F32, tag="psd")
                mm(psd[:], ones4[pc:pc + 1, :], dsl, start=True, stop=True,
                                 tile_position=(pc, 0))
                cap = pwork.tile([64, 128], F32, tag="cap")
                nc.vector.tensor_tensor(cap[:], psd[:], sh_new[:, 128:256], OP.mult)
                nc.vector.tensor_tensor(h2acc[:], h2acc[:], cap[:], OP.add)

            sh_prev = sh_new
            gc_cur = gc_next

        # ---------- FC + sigmoid head ----------
        pfc = ppsd.tile([1, 128], F32, tag="pfc")
        mm(pfc[:], fct[:], h2acc[:], start=True, stop=True)
        osb = pwork.tile([1, 128], F32, tag="osb")
        nc.scalar.activation(osb[:], pfc[:], AF.Sigmoid, bias=fcb[:, 0:1])
        nc.sync.dma_start(d_out.ap()[:], osb[:])

    nc.compile()
    return nc


def _get_program(t_steps: int):
    if t_steps not in _BUILT:
        _BUILT[t_steps] = _build_program(t_steps)
    return _BUILT[t_steps]


def _prep_core_inputs(x, dmask, weights, t_steps):
    """Host-side layout prep for one core's shard. x: [BL, T, I], dmask: [BL, T]."""
    tq = t_steps // 4
    # xs[p, c] = x[b, t, i] at p=(t%4)*32+i, c=(t//4)*128+b
    xpad = np.zeros((BL, t_steps, 32), np.float32)
    xpad[:, :, :I] = x
    xs = (
        xpad.transpose(1, 2, 0)           # [t, i32, b]
        .reshape(tq, 4, 32, BL)
        .transpose(1, 2, 0, 3)            # [t%4, i32, t//4, b]
        .reshape(128, tq * 128)
    )
    xs = np.ascontiguousarray(xs)
    # ds[p, c] = d[b, t] at p=(t%4)*32, c=(t//4)*128+b
    dsb = np.zeros((128, tq * 128), np.float32)
    dv = (
        dmask.transpose(1, 0)             # [t, b]
        .reshape(tq, 4, BL)
        .transpose(1, 0, 2)               # [t%4, t//4, b]
        .reshape(4, tq * 128)
    )
    dsb[0::32][:4] = dv
    return dict(xs=xs, ds=dsb, **weights)


def _host_weights(w_ih0, w_hh0, b_ih0, b_hh0,
                  w_ih1, w_hh1, b_ih1, b_hh1, fc_w, fc_b):
    def lt(a):  # lhsT helper
        return np.ascontiguousarray(np.asarray(a, np.float32).T)

    def pad32x4(a16):  # [16, 128] -> [128, 128]: zero-pad to 32 rows, tile 4x
        out = np.zeros((32, 128), np.float32)
        out[:16] = a16
        return np.tile(out, (4, 1))

    b0 = np.asarray(b_ih0, np.float32) + np.asarray(b_hh0, np.float32)
    b1 = np.asarray(b_ih1, np.float32) + np.asarray(b_hh1, np.float32)

    def og(a):  # reorder [4H, K] gate rows -> [o; g] stacked
        return np.concatenate([a[3 * H:4 * H], a[2 * H:3 * H]], axis=0)

    sel_row0 = np.concatenate([np.ones((1, 128), np.float32),
                               np.zeros((1, 128), np.float32)], axis=1)
    sel_row1 = np.concatenate([np.zeros((1, 128), np.float32),
                               np.ones((1, 128), np.float32)], axis=1)
    weights = dict(
        wif0=pad32x4(lt(np.asarray(w_ih0)[0:2 * H])),
        wog0=pad32x4(lt(og(np.asarray(w_ih0)))),
        whif0=lt(np.asarray(w_hh0)[0:2 * H]),
        whog0=lt(og(np.asarray(w_hh0))),
        wif1=lt(np.asarray(w_ih1)[0:2 * H]),
        wog1=lt(og(np.asarray(w_ih1))),
        whif1=lt(np.asarray(w_hh1)[0:2 * H]),
        whog1=lt(og(np.asarray(w_hh1))),
        bif=np.stack([b0[0:2 * H], b1[0:2 * H]]).astype(np.float32),
        bog=np.stack([
            np.concatenate([b0[3 * H:4 * H], b0[2 * H:3 * H]]),
            np.concatenate([b1[3 * H:4 * H], b1[2 * H:3 * H]]),
        ]).astype(np.float32),
        sel=np.concatenate([sel_row0, sel_row1]).astype(np.float32),
        sell1=np.concatenate([np.zeros((1, 256), np.float32), sel_row1]).astype(np.float32),
        fct=np.ascontiguousarray(np.asarray(fc_w, np.float32).reshape(1, H).T),
        fcb=np.asarray(fc_b, np.float32).reshape(1, 1),
        ones4=np.ones((128, 64), np.float32),
        zini=np.zeros((64, 256), np.float32),
    )
    return weights


def _run(x, mask, w_ih0, w_hh0, b_ih0, b_hh0,
         w_ih1, w_hh1, b_ih1, b_hh1, fc_w, fc_b, trace=False):
    t_steps = x.shape[1]
    x = np.asarray(x, np.float32)
    mask = np.asarray(mask)

    # d[b, t] = mask[b, t] - mask[b, t+1]  (one-hot at t = len_b - 1)
    m = mask.astype(np.float32)
    d = m - np.concatenate([m[:, 1:], np.zeros((m.shape[0], 1), np.float32)], axis=1)

    weights = _host_weights(w_ih0, w_hh0, b_ih0, b_hh0,
                            w_ih1, w_hh1, b_ih1, b_hh1, fc_w, fc_b)

    nc = _get_program(t_steps)
    in_maps = []
    for c in range(NCORES):
        sl = slice(c * BL, (c + 1) * BL)
        in_maps.append(_prep_core_inputs(x[sl], d[sl], weights, t_steps))

    res = bass_utils.run_bass_kernel_spmd(nc, in_maps, core_ids=list(range(NCORES)),
                                          trace=trace)
    out = np.concatenate([res.results[c]["out"].reshape(BL) for c in range(NCORES)])
    return out.astype(np.float32), res


def kernel(**inputs):
    return _run(**inputs)[0]


def kernel_traced(**inputs):
    return _run(**inputs, trace=True)
